# revision 46
# baseline (speedup 1.0000x reference)
"""DCNv2 (modulated deformable conv 3x3 + BatchNorm + SiLU) on Trainium2.

Full problem: x[4,256,80,80]; offset/mask conv (256->27); bilinear-sampled
modulated deformable conv (256->256); BN (batch stats); SiLU.

SPMD over 8 NeuronCores: shard = (batch, row-half) = 1 batch x 40 rows.

Per-core pipeline:
  B. offset/mask conv via 18 shifted matmuls (bf16 in, fp32 accum),
     chunked by row-blocks so later phases overlap.
  C. PE-transpose om into pixel-on-partition layout (pixel column w = partition).
  D. DVE coordinate math -> 4 bilinear corner weights (mask folded in) and
     local-scatter indices per (pixel, tap, corner, column-shift).
     Candidates are re-partitioned (partition = source column w' = p - d') via
     PE shift-matmuls (shifted identities as lhsT; bf16 for weights, fp16 for
     indices, which are exact up to 2048).  Shift matrices are zero outside
     [0,W) so edge partitions come out 0 -> index -1 (ignored by scatter).
  E. Per output row j and tap-row group g (3 taps sharing ky):
     GPSIMD local_scatter builds the selection matrix
        S_g[w'(part), drow-local, tloc, p] (7x3x80 = 1680 slots, auto-zeroed)
     and the tensor engine computes the gather+bilinear+mask as
        val[c, (t,p)] += sum_{drow} xwin[row, w', c] * S_g[w', ...]
     accumulating 7 window rows in PSUM.  The x window lives in a 12-slot
     ring buffer (one 64KB row DMA per iteration).  Every RB rows the main
     GEMM y[o,p] += W[o,(c,t)] val[(c,t),p] accumulates in PSUM.
     PSUM->SBUF copies ride the scalar engine (ACT), with BN partial sums
     fused into the y copies via accum_out.
  F. BN partial stats -> AllReduce over the 8 cores.
  G. y = silu(a*y_gemm + b) on ACT; DMA out.
"""

import dataclasses
import numpy as np

import concourse.bacc as bacc
import concourse.bass as bass
import concourse.tile as tile
from concourse import mybir
from concourse.masks import make_identity

F32 = mybir.dt.float32
BF16 = mybir.dt.bfloat16
FP16 = mybir.dt.float16
I32 = mybir.dt.int32
I16 = mybir.dt.int16
ALU = mybir.AluOpType
ACTF = mybir.ActivationFunctionType
BF16_NP = mybir.dt.np(BF16)
AX = mybir.AxisListType


@dataclasses.dataclass(frozen=True)
class Cfg:
    C: int = 256
    CO: int = 256
    H: int = 80
    W: int = 80
    NR: int = 40            # output rows per shard
    B_total: int = 4
    n_cores: int = 8
    M: int = 4              # sampling margin rows: floor(off) in [-M+1, M-2]
    RB: int = 8             # rows per main-GEMM block
    OMT: int = 5            # om-conv row-tile
    NCH: int = 2            # setup chunks
    WINB: int = 12          # x window ring slots
    eps: float = 1e-5
    use_collective: bool = True
    use_silu: bool = True

    T: int = 9
    KY: int = 3             # tap-row groups
    TL: int = 3             # taps per group

    @property
    def CC(self): return self.C // 128

    @property
    def OC(self): return self.CO // 128

    @property
    def WIN(self): return 2 * self.M + 1          # window rows (9)

    @property
    def GW(self): return 2 * self.M - 1           # per-group window rows (7)

    @property
    def ND(self): return 2 * self.M + 1           # column shifts d' (9)

    @property
    def NTP(self): return self.T * self.W         # val cols, tap-major

    @property
    def GSLOT(self): return self.GW * self.TL * self.W   # local_scatter dst (1680)

    @property
    def NCAND(self): return self.ND * self.TL * 4        # candidates/group (108)

    @property
    def PMROWS(self): return self.NR + 2 * self.M        # x_pm rows (48)

    @property
    def bn_count(self): return float(self.B_total * self.H * self.W)


CFG = Cfg()


def _row_tiles(nr, maxrows):
    out, j = [], 0
    while j < nr:
        out.append((j, min(maxrows, nr - j)))
        j += maxrows
    return out


def build_nc(cfg: Cfg = CFG, debug: bool = False):
    nc = bacc.Bacc("TRN2", target_bir_lowering=False,
                   num_devices=cfg.n_cores if cfg.use_collective else None)
    C, CO, H, W, NR, T, M = cfg.C, cfg.CO, cfg.H, cfg.W, cfg.NR, cfg.T, cfg.M
    CC, OC, WIN, GW, ND = cfg.CC, cfg.OC, cfg.WIN, cfg.GW, cfg.ND
    KY, TL, NTP, RB = cfg.KY, cfg.TL, cfg.NTP, cfg.RB
    WINB = cfg.WINB
    NP = NR * W
    XW = W + 2
    GSL = TL * W
    NPM = cfg.PMROWS

    # ---------------- I/O ----------------
    x_cm = nc.dram_tensor("x_cm", [128, CC * (NR + 2) * XW], BF16, kind="ExternalInput")
    x_pm = nc.dram_tensor("x_pm", [NPM * 128, C], BF16, kind="ExternalInput")
    w_om_l = nc.dram_tensor("w_om_l", [128, T * CC * 32], BF16, kind="ExternalInput")
    b_om_t = nc.dram_tensor("b_om_t", [32, 1], F32, kind="ExternalInput")
    w_ct_t = nc.dram_tensor("w_ct_t", [128, T * CC * CO], BF16, kind="ExternalInput")
    ybase_t = nc.dram_tensor("ybase_t", [W, NR * T], F32, kind="ExternalInput")
    xbase_t = nc.dram_tensor("xbase_t", [W, NR * T], F32, kind="ExternalInput")
    ibc0_t = nc.dram_tensor("ibc0_t", [W, NR * T], F32, kind="ExternalInput")
    wconst_t = nc.dram_tensor("wconst_t", [W, NR * T], F32, kind="ExternalInput")
    gb_t = nc.dram_tensor("gb_t", [128, 2 * OC], F32, kind="ExternalInput")

    y_out = nc.dram_tensor("y_out", [128, OC * NP], F32, kind="ExternalOutput")

    if cfg.use_collective:
        cc_in = nc.dram_tensor("cc_in", [128, 2 * OC], F32)
        cc_out = nc.dram_tensor("cc_out", [128, 2 * OC], F32, addr_space="Shared")

    with tile.TileContext(nc) as tc:
        with (
            tc.tile_pool(name="const", bufs=1) as cp,
            tc.tile_pool(name="psa", bufs=6, space="PSUM") as ps_a,
            tc.tile_pool(name="psb", bufs=2, space="PSUM") as ps_b,
        ):
            p1cm = tc.tile_pool(name="ph1", bufs=1)
            p1 = p1cm.__enter__()
            # ---------------- constants ----------------
            xc = p1.tile([128, CC, (NR + 2) * XW], BF16)
            for ci in range(CC):
                nc.sync.dma_start(
                    xc[:, ci, :],
                    x_cm[:, :].rearrange("p (c n) -> p c n", c=CC)[:, ci, :])
            woml = p1.tile([128, T, CC, 32], BF16)
            nc.sync.dma_start(woml[:, :, :, :],
                              w_om_l[:, :].rearrange("p (t c o) -> p t c o", t=T, c=CC))
            bom = p1.tile([32, 1], F32)
            nc.sync.dma_start(bom[:, :], b_om_t[:, :])
            wct = cp.tile([128, T * CC, CO], BF16)
            nc.sync.dma_start(wct[:, :, :],
                              w_ct_t[:, :].rearrange("p (k o) -> p k o", k=T * CC))

            def load_const(name, dram):
                t_ = p1.tile([128, NR, T], F32, name=name, tag=name)
                nc.sync.dma_start(t_[0:W, :, :],
                                  dram[:, :].rearrange("p (r t) -> p r t", t=T))
                return t_

            ybase = load_const("ybase", ybase_t)
            xbase = load_const("xbase", xbase_t)
            ibc0 = load_const("ibc0", ibc0_t)
            wconst = load_const("wconst", wconst_t)
            gb = cp.tile([128, 2 * OC], F32)
            nc.sync.dma_start(gb[:, :], gb_t[:, :])
            ident = p1.tile([128, 128], F32)
            make_identity(nc, ident[:, :])
            # shifted identities, built on-device: sh[p, dpi, :] = ident[p-d', :]
            identb = p1.tile([128, 128], BF16)
            nc.vector.tensor_copy(identb[:, :], ident[:, :])
            identh = p1.tile([128, 128], FP16)
            nc.vector.tensor_copy(identh[:, :], ident[:, :])
            sh_bf = p1.tile([128, ND, 128], BF16)
            nc.vector.memset(sh_bf[:, :, :], 0.0)
            sh_fh = p1.tile([128, ND, 128], FP16)
            nc.vector.memset(sh_fh[:, :, :], 0.0)
            for dpi in range(ND):
                d = dpi - M
                p_lo, p_hi = max(0, d), min(W, W + d)
                nc.sync.dma_start(sh_bf[p_lo:p_hi, dpi, :], identb[p_lo - d:p_hi - d, :])
                nc.sync.dma_start(sh_fh[p_lo:p_hi, dpi, :], identh[p_lo - d:p_hi - d, :])

            # persistent setup outputs
            omt = p1.tile([128, NR, 27], F32)
            w4p = p1.tile([128, 4, NR, T], BF16)
            idxcand = p1.tile([128, ND, 4, NR, T], FP16)
            data_sb = cp.tile([128, NR, KY, ND, TL, 4], BF16)
            idx_sb = cp.tile([128, NR, KY, ND, TL, 4], I16)
            # row-loop buffers in the persistent pool: no aliasing with setup
            # scratch, so the window ring and early scatters/matmuls are not
            # blocked behind setup WAR dependencies
            winr = cp.tile([128, cfg.WINB, C], BF16)
            s_pairs = [cp.tile([128, 2, KY, GW, TL, W], BF16, name=f"sp{i}")
                       for i in range(2)]

            # ---------------- B: offset/mask conv ----------------
            om_sb = p1.tile([32, NP // 2], F32)  # one chunk, reused
            xcv = xc[:, :, :].rearrange("p c (r q) -> p c r q", q=XW)

            def om_conv(j0, j1):
                for (jt, nrt) in _row_tiles(j1 - j0, cfg.OMT):
                    jb = j0 + jt
                    pt = ps_b.tile([32, cfg.OMT * W], F32, tag="mm")
                    n = nrt * W
                    out_ap = pt[:27, 0:n].rearrange("p (r w) -> p r w", w=W)
                    first = True
                    for t in range(T):
                        ky, kx = t // 3, t % 3
                        for ci in range(CC):
                            rhs = xcv[:, ci, jb + ky:jb + ky + nrt, kx:kx + W]
                            nc.tensor.matmul(
                                out_ap, lhsT=woml[:, t, ci, 0:27], rhs=rhs,
                                start=first, stop=(t == T - 1 and ci == CC - 1))
                            first = False
                    lb = (jb - j0) * W
                    nc.scalar.activation(om_sb[0:27, lb:lb + n], pt[:27, 0:n],
                                         ACTF.Identity, bias=bom[0:27, :])
                # C: om -> pixel-on-partition (partitions 0..W)
                for j in range(j0, j1):
                    lb = (j - j0) * W
                    ptt = ps_a.tile([128, 32], F32, tag="sel")
                    nc.tensor.transpose(ptt[0:W, 0:27], om_sb[0:27, lb:lb + W],
                                        ident[0:27, 0:27])
                    nc.scalar.activation(omt[0:W, j, :], ptt[0:W, 0:27], ACTF.Identity)

            # ---------------- D: coordinate math (partitions 0..W) ------
            # flat [128, NR*T] intermediates, chunked by row range so the
            # vector-engine math overlaps the om conv / shift matmuls
            sl = slice(0, W)
            NT = NR * T
            shp = [128, NT]

            def fl(ap3):
                return ap3.rearrange("p a b -> p (a b)")

            def tt(dst, a, b, op, eng=nc.vector):
                eng.tensor_tensor(dst, a, b, op=op)

            def tsc(dst, a, s1, s2, op0, op1=None, eng=nc.vector):
                if op1 is None:
                    eng.tensor_scalar(dst, a, s1, None, op0=op0)
                else:
                    eng.tensor_scalar(dst, a, s1, s2, op0=op0, op1=op1)

            def stt(dst, a, s, b, op0, op1, eng=nc.vector):
                eng.scalar_tensor_tensor(dst, a, s, b, op0=op0, op1=op1)

            def mk(tag):
                return p1.tile(shp, F32, tag=tag, name=tag)

            ys, xs = mk("ys"), mk("xs")
            ti = p1.tile(shp, I32, tag="ti")
            tf, g = mk("tf"), mk("g")
            y0, x0 = mk("y0"), mk("x0")
            ay, ax_ = mk("ay"), mk("ax")
            # aliases: y1/x1 overwrite ys/xs (dead after ay/ax); the clamps
            # run in place, so y0c==y0, y1c==y1==ys, x0c==x0, x1c==x1==xs
            y1, x1 = ys, xs
            y0c, y1c, x0c, x1c = y0, y1, x0, x1
            msk = mk("msk")
            vy0, vy1, vx0, vx1 = mk("vy0"), mk("vy1"), mk("vx0"), mk("vx1")
            a0, a1 = mk("a0"), mk("a1")
            # more aliases for dead values: vm? overwrite vx? in place,
            # b? overwrite vy? (read for a? beforehand), ibp1_i overwrite wf_i
            vm0, vm1 = vx0, vx1
            b0, b1 = vy0, vy1
            wf = [mk(f"wf{i}") for i in range(4)]
            ib0, ib1 = mk("ib0"), mk("ib1")
            dca0, dca1 = mk("dca0"), mk("dca1")
            ibp1 = wf
            nzv, nzg = mk("nzv"), mk("nzg")
            selg = mk("selg")

            def sigmoid_chunk(j0, j1):
                # early, so it isn't queued behind the shift-phase copies
                # on the scalar engine (which would stall the coord math)
                cs = slice(j0 * T, j1 * T)
                nc.scalar.activation(msk[sl, cs], omt[sl, j0:j1, 2 * T:3 * T],
                                     ACTF.Sigmoid)

            def coord(j0, j1):
                cs = slice(j0 * T, j1 * T)
                jsl = slice(j0, j1)
                dy = omt[sl, jsl, 0:2 * T:2]
                dx = omt[sl, jsl, 1:2 * T:2]
                tt(ys[sl, cs], dy, fl(ybase[sl, jsl, :]), ALU.add)
                tt(xs[sl, cs], dx, fl(xbase[sl, jsl, :]), ALU.add)

                def floor_(dst, src):
                    nc.vector.tensor_copy(ti[sl, cs], src)
                    nc.vector.tensor_copy(tf[sl, cs], ti[sl, cs])
                    tt(g[sl, cs], tf[sl, cs], src, ALU.is_gt)
                    tt(dst, tf[sl, cs], g[sl, cs], ALU.subtract)

                floor_(y0[sl, cs], ys[sl, cs])
                floor_(x0[sl, cs], xs[sl, cs])
                tt(ay[sl, cs], ys[sl, cs], y0[sl, cs], ALU.subtract)
                tt(ax_[sl, cs], xs[sl, cs], x0[sl, cs], ALU.subtract)
                tsc(y1[sl, cs], y0[sl, cs], 1.0, None, ALU.add)
                tsc(x1[sl, cs], x0[sl, cs], 1.0, None, ALU.add)

                def valid(dst, src, hi):
                    tsc(g[sl, cs], src, 0.0, None, ALU.is_ge)
                    stt(dst, src, float(hi), g[sl, cs], ALU.is_le, ALU.mult)

                valid(vy0[sl, cs], y0[sl, cs], H - 1)
                valid(vy1[sl, cs], y1[sl, cs], H - 1)
                valid(vx0[sl, cs], x0[sl, cs], W - 1)
                valid(vx1[sl, cs], x1[sl, cs], W - 1)

                def clamp(dst, src, lo, hi):
                    tsc(dst, src, float(lo), float(hi), ALU.max, ALU.min)

                clamp(y0c[sl, cs], y0[sl, cs], 0, H - 1)
                clamp(y1c[sl, cs], y1[sl, cs], 0, H - 1)
                clamp(x0c[sl, cs], x0[sl, cs], 0, W - 1)
                clamp(x1c[sl, cs], x1[sl, cs], 0, W - 1)

                # corner weights (mask folded): vm? = vx? * msk
                tsc(a0[sl, cs], ay[sl, cs], -1.0, 1.0, ALU.mult, ALU.add)
                tt(a0[sl, cs], a0[sl, cs], vy0[sl, cs], ALU.mult)
                tt(a1[sl, cs], ay[sl, cs], vy1[sl, cs], ALU.mult)
                tt(vm0[sl, cs], vx0[sl, cs], msk[sl, cs], ALU.mult)
                tt(vm1[sl, cs], vx1[sl, cs], msk[sl, cs], ALU.mult)
                tsc(b0[sl, cs], ax_[sl, cs], -1.0, 1.0, ALU.mult, ALU.add)
                tt(b0[sl, cs], b0[sl, cs], vm0[sl, cs], ALU.mult)
                tt(b1[sl, cs], ax_[sl, cs], vm1[sl, cs], ALU.mult)

                for (u, v, i, eng) in [(a0, b0, 0, nc.vector), (a0, b1, 1, nc.gpsimd),
                                       (a1, b0, 2, nc.vector), (a1, b1, 3, nc.gpsimd)]:
                    tt(wf[i][sl, cs], u[sl, cs], v[sl, cs], ALU.mult, eng=eng)
                    eng.tensor_copy(fl(w4p[sl, i, jsl, :]), wf[i][sl, cs])

                # ib+1 = y?c*GSL + ibc0 (host folds (M-rj)*GSL + tloc*W + p + 1)
                stt(ib0[sl, cs], y0c[sl, cs], float(GSL), fl(ibc0[sl, jsl, :]),
                    ALU.mult, ALU.add)
                clamp(ib0[sl, cs], ib0[sl, cs], 1, GW * GSL)
                stt(ib1[sl, cs], y1c[sl, cs], float(GSL), fl(ibc0[sl, jsl, :]),
                    ALU.mult, ALU.add)
                clamp(ib1[sl, cs], ib1[sl, cs], 1, GW * GSL)
                # dcadj = p - x?c: candidate is valid for shift d' iff dcadj == d'
                tt(dca0[sl, cs], fl(wconst[sl, jsl, :]), x0c[sl, cs], ALU.subtract)
                tt(dca1[sl, cs], fl(wconst[sl, jsl, :]), x1c[sl, cs], ALU.subtract,
                   eng=nc.gpsimd)
                # ibp1_i = (wf_i != 0) * (ib+1); corners (1,3) multiply on gpsimd
                for (ib, i, nz, eng) in [(ib0, 0, nzv, nc.vector),
                                         (ib0, 1, nzg, nc.gpsimd),
                                         (ib1, 2, nzv, nc.vector),
                                         (ib1, 3, nzg, nc.gpsimd)]:
                    tsc(nz[sl, cs], wf[i][sl, cs], 0.0, None, ALU.not_equal)
                    tt(ibp1[i][sl, cs], nz[sl, cs], ib[sl, cs], ALU.mult, eng=eng)

                # idxcand = (dca == d') * ibp1; x0-corners fused on vector,
                # x1-corners via a compare (vector) + two gpsimd multiplies
                for dpi in range(ND):
                    dp = float(dpi - M)
                    for i in (0, 2):
                        stt(fl(idxcand[sl, dpi, i, jsl, :]), dca0[sl, cs], dp,
                            ibp1[i][sl, cs], ALU.is_equal, ALU.mult)
                    tsc(selg[sl, cs], dca1[sl, cs], dp, None, ALU.is_equal)
                    for i in (1, 3):
                        tt(fl(idxcand[sl, dpi, i, jsl, :]), selg[sl, cs],
                           ibp1[i][sl, cs], ALU.mult, eng=nc.gpsimd)

            def shifts(j0, j1):
                jc = j1 - j0
                for gky in range(KY):
                    for dpi in range(ND):
                        psw = ps_a.tile([128, 4 * jc * TL], F32, tag="sel")
                        pswv = psw[:, :].rearrange("p (c j t) -> p c j t", c=4, j=jc)
                        nc.tensor.matmul(
                            pswv, lhsT=sh_bf[0:W, dpi, :],
                            rhs=w4p[0:W, :, j0:j1, gky * TL:(gky + 1) * TL],
                            start=True, stop=True)
                        nc.scalar.activation(
                            data_sb[:, j0:j1, gky, dpi, :, :],
                            pswv.rearrange("p c j t -> p j t c"),
                            ACTF.Identity)
                        psi = ps_a.tile([128, 4 * jc * TL], F32, tag="sel")
                        psiv = psi[:, :].rearrange("p (c j t) -> p c j t", c=4, j=jc)
                        nc.tensor.matmul(
                            psiv, lhsT=sh_fh[0:W, dpi, :],
                            rhs=idxcand[0:W, dpi, :, j0:j1, gky * TL:(gky + 1) * TL],
                            start=True, stop=True)
                        nc.vector.tensor_scalar(
                            idx_sb[:, j0:j1, gky, dpi, :, :],
                            psiv.rearrange("p c j t -> p j t c"),
                            -1.0, None, op0=ALU.add)

            H2 = NR // 2
            om_conv(0, H2)
            om_conv(H2, NR)
            sigmoid_chunk(0, H2)
            sigmoid_chunk(H2, NR)
            coord(0, H2)
            shifts(0, H2)
            coord(H2, NR)
            shifts(H2, NR)

            # ---------------- E: per-row pipeline ----------------
            p1cm.__exit__(None, None, None)
            p2cm = tc.tile_pool(name="ph2", bufs=1)
            p2 = p2cm.__enter__()
            val_sb = p2.tile([128, CC, RB, NTP], BF16)
            y_sb = p2.tile([128, OC, NP], F32)
            sq_sc = p2.tile([128, RB * W], F32)
            nblk = -(-NR // RB)
            nsp = -(-(RB * W) // 512)
            wsp = -(-W // nsp)
            # BN partial sums: [sum | sumsq] x oi x (block, split)
            nparts = nblk * nsp
            parts = cp.tile([128, 2, OC, nparts], F32)

            # window ring prologue: rows 0..WIN+1 (one pair of prefetch lead)
            nc.sync.dma_start(
                winr[:, 0:WIN + 2, :],
                x_pm[0:(WIN + 2) * 128, :].rearrange("(k p) c -> p k c", p=128))

            GSLOT = cfg.GSLOT
            for jp in range(NR // 2):
                j0 = jp * 2
                for nxt in (j0 + WIN + 2, j0 + WIN + 3):
                    if nxt < NPM:
                        nc.sync.dma_start(winr[:, nxt % WINB, :],
                                          x_pm[nxt * 128:(nxt + 1) * 128, :])
                # S matrices for both rows of the pair in one tile so a single
                # matmul can span (row0, drl=drp) and (row1, drl=drp-1): those
                # share the same absolute window row, hence the same lhsT.
                s_pair = s_pairs[jp % 2]
                for r in range(2):
                    for gky in range(KY):
                        nc.gpsimd.local_scatter(
                            out_ap=s_pair[:, r, gky, :, :, :].rearrange(
                                "p a b c -> p (a b c)"),
                            data_ap=data_sb[:, j0 + r, gky, :, :, :].rearrange(
                                "p a b c -> p (a b c)"),
                            idxs_ap=idx_sb[:, j0 + r, gky, :, :, :].rearrange(
                                "p a b c -> p (a b c)"),
                            channels=128,
                            num_elems=GSLOT,
                            num_idxs=cfg.NCAND,
                        )
                sp_full = s_pair[:, :, :, :, :, :]
                sp_pstride = sp_full.ap[0][0]
                pv = {}
                for gky in range(KY):
                    for ci in range(CC):
                        pv[(gky, ci)] = ps_a.tile([128, 2, TL * W], F32,
                                                  tag="sel", name="pv")
                for dr_abs in range(WIN + 1):
                    for ci in range(CC):
                        lhsT = winr[:, (j0 + dr_abs) % WINB, ci * 128:(ci + 1) * 128]
                        for gky in range(KY):
                            drp = dr_abs - gky
                            if not (0 <= drp <= GW):
                                continue
                            p_ = pv[(gky, ci)]
                            if drp < GW and drp >= 2:
                                # wide: both rows in one matmul
                                rhs = bass.AP(
                                    sp_full.tensor,
                                    sp_full.offset + gky * GSLOT + drp * GSL,
                                    [[sp_pstride, 128],
                                     [KY * GSLOT - GSL, 2], [1, GSL]])
                                nc.tensor.matmul(
                                    p_[:, :, :], lhsT=lhsT, rhs=rhs,
                                    start=False, stop=False,
                                    skip_group_check=True)
                            else:
                                if drp < GW:  # row0 at drl=drp (drp 0 or 1)
                                    nc.tensor.matmul(
                                        p_[:, 0, :], lhsT=lhsT,
                                        rhs=s_pair[:, 0, gky, drp, :, :].rearrange(
                                            "p a b -> p (a b)"),
                                        start=(drp == 0), stop=False,
                                        skip_group_check=True)
                                if drp >= 1:  # row1 at drl=drp-1 (drp 1 or 7)
                                    nc.tensor.matmul(
                                        p_[:, 1, :], lhsT=lhsT,
                                        rhs=s_pair[:, 1, gky, drp - 1, :, :].rearrange(
                                            "p a b -> p (a b)"),
                                        start=False, stop=(drp == GW),
                                        skip_group_check=True)
                for gky in range(KY):
                    for ci in range(CC):
                        for r in range(2):
                            nc.scalar.activation(
                                val_sb[:, ci, (j0 + r) % RB,
                                       gky * TL * W:(gky + 1) * TL * W],
                                pv[(gky, ci)][:, r, :], ACTF.Identity)

                j = j0 + 1
                if (j + 1) % RB == 0 or j == NR - 1:
                    rbeg = (j // RB) * RB
                    rcnt = j - rbeg + 1
                    blk = j // RB
                    valv = val_sb[:, :, :, :].rearrange("p c r (t w) -> p c r t w", t=T)
                    for oi in range(OC):
                        for s in range(nsp):
                            w0 = s * wsp
                            w1 = min(w0 + wsp, W)
                            py = ps_b.tile([128, RB * wsp], F32, tag="mm")
                            out_ap = py[:, 0:rcnt * (w1 - w0)].rearrange(
                                "p (r w) -> p r w", w=w1 - w0)
                            first = True
                            for t in range(T):
                                for ci in range(CC):
                                    nc.tensor.matmul(
                                        out_ap,
                                        lhsT=wct[:, t * CC + ci, oi * 128:(oi + 1) * 128],
                                        rhs=valv[:, ci, 0:rcnt, t, w0:w1],
                                        start=first,
                                        stop=(t == T - 1 and ci == CC - 1))
                                    first = False
                            ydst = y_sb[:, oi, rbeg * W:(rbeg + rcnt) * W].rearrange(
                                "p (r w) -> p r w", w=W)[:, :, w0:w1]
                            pidx = blk * nsp + s
                            nc.scalar.activation(
                                ydst, out_ap, ACTF.Identity,
                                accum_out=parts[:, 0, oi, pidx:pidx + 1])
                            nc.scalar.activation(
                                sq_sc[:, 0:rcnt * (w1 - w0)],
                                py[:, 0:rcnt * (w1 - w0)], ACTF.Square,
                                accum_out=parts[:, 1, oi, pidx:pidx + 1])

            # ---------------- F: BN stats (+ allreduce) ----------------
            stats = cp.tile([128, 2 * OC], F32)
            for oi in range(OC):
                nc.vector.tensor_reduce(stats[:, oi:oi + 1],
                                        parts[:, 0, oi, :], axis=AX.X, op=ALU.add)
                nc.vector.tensor_reduce(stats[:, OC + oi:OC + oi + 1],
                                        parts[:, 1, oi, :], axis=AX.X, op=ALU.add)
            if cfg.use_collective:
                nc.sync.dma_start(cc_in[:, :], stats[:, :])
                nc.gpsimd.collective_compute(
                    "AllReduce", ALU.add,
                    replica_groups=[list(range(cfg.n_cores))],
                    ins=[cc_in[:, :]], outs=[cc_out[:, :]])
                nc.sync.dma_start(stats[:, :], cc_out[:, :])

            # ---------------- G: affine + SiLU ----------------
            cnt = cfg.bn_count
            mean = cp.tile([128, OC], F32)
            var = cp.tile([128, OC], F32)
            aa = cp.tile([128, OC], F32)
            bb = cp.tile([128, OC], F32)
            sqm = cp.tile([128, 1], F32)
            # halved output staging (2 activation+DMA slices per oi)
            out_t = p2.tile([128, 2, NP // 2], F32)
            for oi in range(OC):
                nc.vector.tensor_scalar(mean[:, oi:oi + 1], stats[:, oi:oi + 1],
                                        1.0 / cnt, None, op0=ALU.mult)
                nc.vector.tensor_scalar(var[:, oi:oi + 1], stats[:, OC + oi:OC + oi + 1],
                                        1.0 / cnt, None, op0=ALU.mult)
                nc.vector.tensor_tensor(sqm[:, 0:1], mean[:, oi:oi + 1], mean[:, oi:oi + 1],
                                        op=ALU.mult)
                nc.vector.tensor_tensor(var[:, oi:oi + 1], var[:, oi:oi + 1], sqm[:, 0:1],
                                        op=ALU.subtract)
                nc.vector.tensor_scalar(var[:, oi:oi + 1], var[:, oi:oi + 1], cfg.eps,
                                        None, op0=ALU.add)
                nc.scalar.sqrt(var[:, oi:oi + 1], var[:, oi:oi + 1])
                nc.vector.reciprocal(var[:, oi:oi + 1], var[:, oi:oi + 1])
                nc.vector.tensor_tensor(aa[:, oi:oi + 1], gb[:, oi:oi + 1],
                                        var[:, oi:oi + 1], op=ALU.mult)
                nc.vector.tensor_tensor(bb[:, oi:oi + 1], mean[:, oi:oi + 1],
                                        aa[:, oi:oi + 1], op=ALU.mult)
                nc.vector.tensor_tensor(bb[:, oi:oi + 1], gb[:, OC + oi:OC + oi + 1],
                                        bb[:, oi:oi + 1], op=ALU.subtract)
                for h in range(2):
                    hs = slice(h * (NP // 2), (h + 1) * (NP // 2))
                    nc.scalar.activation(out_t[:, h, :], y_sb[:, oi, hs], ACTF.Silu,
                                         bias=bb[:, oi:oi + 1], scale=aa[:, oi:oi + 1])
                    nc.sync.dma_start(
                        y_out[:, :].rearrange("p (c n) -> p c n", c=OC)[:, oi, hs],
                        out_t[:, h, :])
            p2cm.__exit__(None, None, None)

    nc.compile()
    return nc


# ======================= host side =======================

def host_prepare(inputs: dict, cfg: Cfg = CFG):
    x = np.asarray(inputs["x"], np.float32)
    w_om = np.asarray(inputs["w_om"], np.float32)
    b_om = np.asarray(inputs["b_om"], np.float32)
    weight = np.asarray(inputs["weight"], np.float32)
    gamma = np.asarray(inputs["gamma"], np.float32)
    beta = np.asarray(inputs["beta"], np.float32)
    # conv bias cancels inside batch-stat BN (shift-invariant) — not needed.
    C, CO, H, W, NR, T, M = cfg.C, cfg.CO, cfg.H, cfg.W, cfg.NR, cfg.T, cfg.M
    CC, OC, TL = cfg.CC, cfg.OC, cfg.TL
    B = x.shape[0]
    halves = max(cfg.n_cores // B, 1)
    XW = W + 2
    GSL = TL * W

    w_om_l = np.zeros((128, T, CC, 32), BF16_NP)
    for t in range(T):
        ky, kx = t // 3, t % 3
        for ci in range(CC):
            w_om_l[:, t, ci, :27] = w_om[:, ci * 128:(ci + 1) * 128, ky, kx].T.astype(BF16_NP)
    b_om_t = np.zeros((32, 1), np.float32)
    b_om_t[:27, 0] = b_om
    w_ct = np.zeros((128, T * CC, CO), BF16_NP)
    for t in range(T):
        ky, kx = t // 3, t % 3
        for ci in range(CC):
            w_ct[:, t * CC + ci, :] = weight[:, ci * 128:(ci + 1) * 128, ky, kx].T.astype(BF16_NP)
    gb = np.zeros((128, 2 * OC), np.float32)
    for oi in range(OC):
        gb[:, oi] = gamma[oi * 128:(oi + 1) * 128]
        gb[:, OC + oi] = beta[oi * 128:(oi + 1) * 128]

    jj, tt_ = np.meshgrid(np.arange(NR), np.arange(T), indexing="ij")
    ky_m = (tt_ // 3).astype(np.float32)
    kx_m = (tt_ % 3).astype(np.float32)
    tloc_m = (tt_ % 3).astype(np.float32)  # within-group tap index (t = 3*ky + kx)
    wcol = np.arange(W, dtype=np.float32).reshape(W, 1, 1)

    in_maps = []
    for core in range(cfg.n_cores):
        b = core // halves
        r0 = (core % halves) * NR
        xcm = np.zeros((128, CC, NR + 2, XW), BF16_NP)
        lo = r0 - 1
        src_lo, src_hi = max(lo, 0), min(r0 + NR + 1, H)
        xs_ = x[b, :, src_lo:src_hi, :]
        for ci in range(CC):
            xcm[:, ci, (src_lo - lo):(src_lo - lo) + xs_.shape[1], 1:1 + W] = \
                xs_[ci * 128:(ci + 1) * 128].astype(BF16_NP)
        xcm = xcm.reshape(128, CC * (NR + 2) * XW)

        # pixel-major x, rows padded to 128 columns, M pad rows top/bottom
        xp = np.zeros((cfg.PMROWS, 128, C), BF16_NP)
        gl_lo = r0 - M
        g0, g1 = max(gl_lo, 0), min(gl_lo + cfg.PMROWS, H)
        xp[g0 - gl_lo:g1 - gl_lo, :W, :] = np.transpose(
            x[b, :, g0:g1, :], (1, 2, 0)).astype(BF16_NP)
        x_pm = xp.reshape(cfg.PMROWS * 128, C)

        rj = (r0 + jj).astype(np.float32)
        ybase = np.broadcast_to((rj + ky_m - 1)[None], (W, NR, T))
        xbase = wcol + np.broadcast_to((kx_m - 1)[None], (W, NR, T))
        # ib+1 = y?c*GSL + ibc0; ibc0 = (M - rj)*GSL - ky*TL*W + tloc*W + p + 1
        ibc0 = ((M - rj) * GSL - ky_m * (TL * W) + tloc_m * W + 1.0)[None] + wcol
        wconst = np.broadcast_to(wcol, (W, NR, T))
        in_maps.append(dict(
            x_cm=xcm,
            x_pm=x_pm,
            w_om_l=w_om_l.reshape(128, T * CC * 32),
            b_om_t=b_om_t,
            w_ct_t=w_ct.reshape(128, T * CC * CO),
            ybase_t=np.ascontiguousarray(ybase, np.float32).reshape(W, NR * T),
            xbase_t=np.ascontiguousarray(xbase, np.float32).reshape(W, NR * T),
            ibc0_t=np.ascontiguousarray(ibc0, np.float32).reshape(W, NR * T),
            wconst_t=np.ascontiguousarray(wconst, np.float32).reshape(W, NR * T),
            gb_t=gb,
        ))
    return in_maps


def reassemble(results, cfg: Cfg = CFG):
    B, halves = cfg.B_total, max(cfg.n_cores // cfg.B_total, 1)
    H, W, NR = cfg.H, cfg.W, cfg.NR
    y = np.zeros((B, cfg.CO, H, W), np.float32)
    for core, res in enumerate(results):
        b = core // halves
        r0 = (core % halves) * NR
        yo = np.asarray(res["y_out"]).reshape(128, cfg.OC, NR, W)
        for oi in range(cfg.OC):
            y[b, oi * 128:(oi + 1) * 128, r0:r0 + NR, :] = yo[:, oi]
    return y


_NC_CACHE = {}


def kernel(**inputs) -> np.ndarray:
    from concourse.bass_utils import run_bass_kernel_spmd
    cfg = CFG
    if "nc" not in _NC_CACHE:
        _NC_CACHE["nc"] = build_nc(cfg)
    nc = _NC_CACHE["nc"]
    in_maps = host_prepare(inputs, cfg)
    res = run_bass_kernel_spmd(nc, in_maps, core_ids=list(range(cfg.n_cores)))
    return reassemble(res.results, cfg)
